# revision 42
# baseline (speedup 1.0000x reference)
"""Local (windowed) attention Trainium2 Bass kernel.

Problem: q,k,v [8, 8, 4096, 64] fp32; window 128, look_backward 1, pad -1.0.
out[b,h,w,i,:] = softmax(scale * q_wi . [k_{w-1}; k_w]) @ [v_{w-1}; v_w]
(with window -1 = all -1.0 pad values, which DO enter the softmax).

Sharding: data-parallel over flat batch*heads (64) -> 8 heads per core.

Per-core layouts (prepared host-side):
  qT : [4, 128, 4096]  float16 - head pair stacked on partitions (d=64 each),
                                 free axis = 4096 queries (d-major transposed)
  kT : [4, 128, 4224]  float16 - same, with one pad chunk (128 keys of -1.0)
                                 prepended -> 33 chunks of 128 keys
  v  : [8, 128, 33, 72] float16 - per head; partition = key-within-chunk,
                                 pad chunk prepended; col 64 = 1.0 (ones
                                 column yields softmax denominator l); cols
                                 65..71 zero pad for 16 B row alignment
  out: [8, 65, 4096] float16 - TRANSPOSED unnormalized output: rows 0..63 =
                               sum_j P[j,q] v[j,:], row 64 = l[q].  The final
                               division by l happens on the host.

Device pipeline per head pair, per key chunk p (0..32):
  MM1 (fp16): scoresT[j, i] for the <=2 windows attending chunk p
              lhsT = kT chunk [64,128], rhs = qT slice [64,256];
              heads of a pair alternate PE row groups (base partition 0/64)
              so LDWEIGHTS pulls ahead under the other head's matmul and the
              row-tiled matmuls run concurrently; each PSUM bank only ever
              sees one weight base partition (mixing row-group bases within
              a bank hard-crashes the device).
  ACT exp (scale=1/8, bias=-3.5) psum -> fp16 P tiles, one [128, 1024]
      ACTIVATE per 2-chunk group (both heads contiguous).  The constant
      bias keeps exp sums in fp16 range; it cancels in the final P/l
      division.  The exp table is pre-loaded during the initial DMA wait.
  MM2 (fp16), operands swapped vs the score layout: stationary = v_aug
      chunk [128 keys, 65] (65-column LDWEIGHTS instead of 128-column P
      loads), moving = P columns.  Output is out^T [65, queries] in
      per-window psum slots [65, 128]: chunk p closes window p-1
      (start=False) and opens window p (start=True).  After compile,
      _dedup_ldweights deletes the second, redundant v LDWEIGHTS of each
      close/open pair straight out of the scheduled instruction stream.
  DVE: evacuate psum slot banks [65, 4 windows, 128] -> fp16 sbuf; DMA out
      every 8 windows.

PSUM budget (8 banks): 3 score buffers x [128, 1024] f32 (2 banks each) for
exp lookahead + 2 out-slot banks.

Accuracy: ~7e-4 relative (fp16 operands and fp16 unnormalized output; exact
fp32 PSUM accumulation; host-side fp32 division).
"""

import os
import sys

for _p in ("/opt/trn_rl_repo", "/opt/pypackages"):
    if os.path.isdir(_p) and _p not in sys.path:
        sys.path.append(_p)

import numpy as np

import concourse.mybir as mybir
import concourse.tile as tile
from concourse import bacc
from concourse.bass_utils import run_bass_kernel_spmd

B, H, N, D = 8, 8, 4096, 64
WS = 128                 # window size
W = N // WS              # 32 windows
C = W + 1                # 33 key chunks incl. pad chunk
NC = 8                   # cores
HPC = (B * H) // NC      # 8 heads per core
PAIRS = HPC // 2         # 4 head pairs per core
SCALE = float(D) ** -0.5
EXP_BIAS = -3.5          # exp(x*SCALE + EXP_BIAS): cancels in P/l, keeps fp16 range

MM_DT = mybir.dt.float16
GROUP = 2                # key chunks per exp batch (h-block must stay 512-col
                         # = one psum bank aligned, so GROUP must be even)
EVW = 4                  # windows per psum out bank
DMW = 8                  # windows per out DMA (2 banks per staging tile)
VP = 72                  # v chunk padded to 72 cols: 144 B rows keep the
                         # 65-col LDWEIGHTS slices 16 B-aligned in SBUF

_NC_CACHE = {}


def build_nc(pairs=PAIRS, w=W):
    c = w + 1
    n = w * WS
    nc = bacc.Bacc("TRN2", target_bir_lowering=False)
    qT = nc.dram_tensor("qT", [pairs, 128, n], MM_DT, kind="ExternalInput")
    kT = nc.dram_tensor("kT", [pairs, 128, c * WS], MM_DT, kind="ExternalInput")
    vv = nc.dram_tensor("v", [2 * pairs, 128, c, VP], MM_DT, kind="ExternalInput")
    out = nc.dram_tensor("out", [2 * pairs, D + 1, n], MM_DT,
                         kind="ExternalOutput")

    f32 = mybir.dt.float32
    ch = c // 2
    Exp = mybir.ActivationFunctionType.Exp

    with tile.TileContext(nc) as tc:
        with (
            tc.tile_pool(name="cst", bufs=1) as cst_pool,
            tc.tile_pool(name="qk", bufs=2) as qk_pool,
            tc.tile_pool(name="vp", bufs=4) as v_pool,
            tc.tile_pool(name="pt", bufs=6) as pt_pool,
            tc.tile_pool(name="st", bufs=6) as st_pool,
            tc.tile_pool(name="ps_s", bufs=3, space="PSUM") as ps_s,
            tc.tile_pool(name="ps_o", bufs=2, space="PSUM") as ps_o,
        ):
            bias_t = cst_pool.tile([128, 1], f32, tag="bias")
            nc.vector.memset(bias_t[:, :], EXP_BIAS)
            # trigger the one-time exp ACT_TABLE_LOAD during the input DMA wait
            warm_t = cst_pool.tile([128, 1], MM_DT, tag="warm")
            nc.scalar.activation(warm_t[:, :], bias_t[:, :], Exp,
                                 bias=bias_t[:, 0:1], scale=SCALE)

            for pair in range(pairs):
                qt = qk_pool.tile([128, n], MM_DT, tag="qT")
                kt = qk_pool.tile([128, c * WS], MM_DT, tag="kT")
                vts = [v_pool.tile([128, c, VP], MM_DT, tag="v",
                                   name=f"v_{pair}_{h}") for h in range(2)]

                NSL = 8 if pair == 0 else 4
                ck, cq = c * WS // NSL, n // NSL

                def load_slice(sl):
                    nc.sync.dma_start(kt[:, sl * ck:(sl + 1) * ck],
                                      kT[pair][:, sl * ck:(sl + 1) * ck])
                    nc.sync.dma_start(qt[:, sl * cq:(sl + 1) * cq],
                                      qT[pair][:, sl * cq:(sl + 1) * cq])

                if pair == 0:
                    # split only the first slice across 8 queues, dispatched
                    # from BOTH hwdge engines (Sync + idle-at-startup ACT):
                    # a single [128, 528] transfer on one ~20 B/ns queue
                    # would gate the first MM1 by ~5us, and 8 dispatches on
                    # one engine cost ~0.7us each serially
                    for i in range(4):
                        nc.sync.dma_start(kt[:, i * 132:(i + 1) * 132],
                                          kT[pair][:, i * 132:(i + 1) * 132])
                        nc.scalar.dma_start(qt[:, i * 128:(i + 1) * 128],
                                            qT[pair][:, i * 128:(i + 1) * 128])
                else:
                    load_slice(0)
                # interleave v halves between input slices: HWDGE DMAs drain
                # FIFO per engine, so a monolithic v load would delay the
                # kt/qt slices that feed the next MM1s
                for h in range(2):
                    nc.sync.dma_start(vts[h][:, 0:ch], vv[2 * pair + h][:, 0:ch])
                load_slice(1)
                for h in range(2):
                    nc.sync.dma_start(vts[h][:, ch:], vv[2 * pair + h][:, ch:])
                for sl in range(2, NSL):
                    load_slice(sl)

                # per-head ring of psum out banks, each holding EVW windows
                banks = [dict(), dict()]   # h -> {bank_idx: psum tile}

                def get_bank(h, wi):
                    bi = wi // EVW
                    if bi not in banks[h]:
                        banks[h][bi] = ps_o.tile(
                            [D + 1, EVW, WS], f32, tag="out",
                            name=f"ob_{pair}_{h}_{bi}")
                    return banks[h][bi]

                stgs = [dict(), dict()]  # h -> {dma_idx: staging tile}

                def evac_bank(h, bi, nwin):
                    t = banks[h].pop(bi)
                    di, half = divmod(bi, DMW // EVW)
                    if di not in stgs[h]:
                        stgs[h][di] = st_pool.tile(
                            [D + 1, DMW, WS], MM_DT, tag="stg",
                            name=f"st_{pair}_{h}_{di}")
                    stg = stgs[h][di]
                    nc.vector.tensor_copy(
                        stg[:, half * EVW:half * EVW + nwin], t[:, 0:nwin])
                    if half * EVW + nwin == DMW or bi * EVW + nwin == w:
                        stgs[h].pop(di)
                        nc.sync.dma_start(
                            out[2 * pair + h][:, di * DMW * WS:
                                              (di * DMW + half * EVW + nwin)
                                              * WS],
                            stg[:, 0:half * EVW + nwin])

                groups = [list(range(g, min(g + GROUP, c)))
                          for g in range(0, c, GROUP)]
                pending_mm2 = None

                def do_mm2s(chunks, pt):
                    for s, p in enumerate(chunks):
                        for h in range(2):
                            col = h * (GROUP * 256) + s * 256
                            vav = vts[h][:, p, 0:D + 1]
                            if p >= 1:
                                # close window p-1 (self-keys contribution);
                                # emitted before the open so the open's bank
                                # recycle never waits on a later instruction
                                wi = p - 1
                                bank = get_bank(h, wi)
                                nc.tensor.matmul(
                                    bank[:, wi % EVW],
                                    vav, pt[:, col:col + WS],
                                    start=False, stop=True,
                                )
                                if (wi + 1) % EVW == 0 or wi == w - 1:
                                    evac_bank(h, wi // EVW, wi % EVW + 1)
                            if p <= w - 1:
                                # open window p (its prev-keys contribution)
                                colB = col + (WS if p >= 1 else 0)
                                bank = get_bank(h, p)
                                nc.tensor.matmul(
                                    bank[:, p % EVW],
                                    vav, pt[:, colB:colB + WS],
                                    start=True, stop=False,
                                )

                for chunks in groups:
                    ps = ps_s.tile([128, GROUP * 2 * 256], f32, tag="scores")
                    runs = []  # written (col, n) regions
                    for s, p in enumerate(chunks):
                        qlo = max(0, (p - 1) * WS)
                        qhi = min(n, (p + 1) * WS)
                        if p == 0:
                            qhi = 2 * WS   # fill the slot: keeps the exp run
                        nq = qhi - qlo     # contiguous (upper half unused)
                        for h in range(2):
                            col = h * (GROUP * 256) + s * 256
                            nc.tensor.matmul(
                                ps[:, col:col + nq],
                                kt[64 * h:64 * h + 64, p * WS:(p + 1) * WS],
                                qt[64 * h:64 * h + 64, qlo:qhi],
                                start=True, stop=True,
                            )
                            runs.append((col, nq))
                    pt = pt_pool.tile([128, GROUP * 2 * 256], MM_DT, tag="pt")
                    merged = []
                    for rcol, rn in sorted(runs):
                        if merged and merged[-1][0] + merged[-1][1] == rcol:
                            merged[-1][1] += rn
                        else:
                            merged.append([rcol, rn])
                    for rcol, rn in merged:
                        nc.scalar.activation(pt[:, rcol:rcol + rn],
                                             ps[:, rcol:rcol + rn],
                                             Exp, bias=bias_t[:, 0:1],
                                             scale=SCALE)
                    if pending_mm2 is not None:
                        do_mm2s(*pending_mm2)
                    pending_mm2 = (chunks, pt)
                if pending_mm2 is not None:
                    do_mm2s(*pending_mm2)
                    pending_mm2 = None

    nc.compile()
    _dedup_ldweights(nc)
    return nc


def _dedup_ldweights(nc):
    """Drop back-to-back identical PE weight loads.

    The close/open MM2 matmuls of a chunk share the same stationary v tile
    but legalization emits one InstLdweights per matmul.  After scheduling,
    wherever two consecutive PE InstLdweights have identical physical access
    patterns (only non-LDW PE instructions between them) the second load is
    redundant - the weights are still resident in the array.  Only sync-free
    duplicates are dropped so no semaphore bookkeeping changes.
    """
    removed = 0
    pe = mybir.EngineType.PE
    for fn in nc.m.functions:
        for blk in fn.blocks:
            il = blk.instructions
            keep = []
            last_key = None
            for inst in il:
                if getattr(inst, "engine", None) == pe:
                    tname = type(inst).__name__
                    if tname == "InstLdweights":
                        key = (repr(inst.ins), inst.tile_position,
                               inst.perf_mode, inst.is_transpose)
                        si = inst.sync_info
                        clean = si is None or (not si.on_wait
                                               and not si.on_update)
                        if key == last_key and clean:
                            removed += 1
                            continue
                        last_key = key
                keep.append(inst)
            if removed:
                il[:] = keep
    if removed:
        print(f"[kernel] deduped {removed} redundant LDWEIGHTS", file=sys.stderr)
    return removed


def _get_nc():
    if "nc" not in _NC_CACHE:
        _NC_CACHE["nc"] = build_nc()
    return _NC_CACHE["nc"]


def _prep_core(qf, kf, vf, lo):
    """Build one core's input dict from flat [64, 4096, 64] fp32 arrays."""
    q8 = qf[lo:lo + HPC]                      # [8, 4096, 64]
    k8 = kf[lo:lo + HPC]
    v8 = vf[lo:lo + HPC]

    qT = np.ascontiguousarray(q8.transpose(0, 2, 1)).reshape(PAIRS, 128, N)
    qT = qT.astype(np.float16)

    pad = np.full((HPC, WS, D), -1.0, dtype=np.float32)
    kp = np.concatenate([pad, k8], axis=1)    # [8, 4224, 64]
    kT = np.ascontiguousarray(kp.transpose(0, 2, 1)).reshape(PAIRS, 128, C * WS)
    kT = kT.astype(np.float16)

    vp = np.concatenate([pad, v8], axis=1)    # [8, 4224, 64]
    ones = np.ones((HPC, C * WS, 1), dtype=np.float32)
    zpad = np.zeros((HPC, C * WS, VP - D - 1), dtype=np.float32)
    va = np.concatenate([vp, ones, zpad], axis=2)   # [8, 4224, 72]
    va = va.reshape(HPC, C, WS, VP).transpose(0, 2, 1, 3)  # [8, 128, 33, 72]
    va = np.ascontiguousarray(va).astype(np.float16)

    return {"qT": qT, "kT": kT, "v": va}


def kernel(q, k, v):
    q = np.asarray(q, dtype=np.float32)
    k = np.asarray(k, dtype=np.float32)
    v = np.asarray(v, dtype=np.float32)
    qf = q.reshape(B * H, N, D)
    kf = k.reshape(B * H, N, D)
    vf = v.reshape(B * H, N, D)

    nc = _get_nc()
    in_maps = [_prep_core(qf, kf, vf, HPC * c) for c in range(NC)]
    res = run_bass_kernel_spmd(nc, in_maps, core_ids=list(range(NC)))

    outs = []
    for c in range(NC):
        o = res.results[c]["out"].astype(np.float32)   # [8, 65, 4096]
        o = o[:, :D, :] / o[:, D:D + 1, :]             # normalize by l row
        outs.append(o.transpose(0, 2, 1))              # [8, 4096, 64]
    return np.concatenate(outs, axis=0).reshape(B, H, N, D).astype(np.float32)


if __name__ == "__main__":
    rng = np.random.default_rng(0)
    q = rng.standard_normal((B, H, N, D), dtype=np.float32)
    k = rng.standard_normal((B, H, N, D), dtype=np.float32)
    v = rng.standard_normal((B, H, N, D), dtype=np.float32)
    o = kernel(q, k, v)
    print("out", o.shape, o.dtype, float(np.abs(o).max()))


# revision 43
# speedup vs baseline: 1.0397x; 1.0397x over previous
"""Local (windowed) attention Trainium2 Bass kernel.

Problem: q,k,v [8, 8, 4096, 64] fp32; window 128, look_backward 1, pad -1.0.
out[b,h,w,i,:] = softmax(scale * q_wi . [k_{w-1}; k_w]) @ [v_{w-1}; v_w]
(with window -1 = all -1.0 pad values, which DO enter the softmax).

Sharding: data-parallel over flat batch*heads (64) -> 8 heads per core.

Per-core layouts (prepared host-side):
  qT : [4, 128, 4096]  float16 - head pair stacked on partitions (d=64 each),
                                 free axis = 4096 queries (d-major transposed)
  kT : [4, 128, 4224]  float16 - same, with one pad chunk (128 keys of -1.0)
                                 prepended -> 33 chunks of 128 keys
  v  : [8, 128, 33, 72] float16 - per head; partition = key-within-chunk,
                                 pad chunk prepended; col 64 = 1.0 (ones
                                 column yields softmax denominator l); cols
                                 65..71 zero pad for 16 B row alignment
  out: [8, 65, 4096] float16 - TRANSPOSED unnormalized output: rows 0..63 =
                               sum_j P[j,q] v[j,:], row 64 = l[q].  The final
                               division by l happens on the host.

Device pipeline per head pair, per key chunk p (0..32):
  MM1 (fp16): scoresT[j, i] for the <=2 windows attending chunk p
              lhsT = kT chunk [64,128], rhs = qT slice [64,256];
              heads of a pair alternate PE row groups (base partition 0/64)
              so LDWEIGHTS pulls ahead under the other head's matmul and the
              row-tiled matmuls run concurrently; each PSUM bank only ever
              sees one weight base partition (mixing row-group bases within
              a bank hard-crashes the device).
  ACT exp (scale=1/8, bias=-3.5) psum -> fp16 P tiles, one [128, 1024]
      ACTIVATE per 2-chunk group (both heads contiguous).  The constant
      bias keeps exp sums in fp16 range; it cancels in the final P/l
      division.  The exp table is pre-loaded during the initial DMA wait.
  MM2 (fp16), operands swapped vs the score layout: stationary = v_aug
      chunk [128 keys, 65] (65-column LDWEIGHTS instead of 128-column P
      loads), moving = P columns.  Output is out^T [65, queries] in
      per-window psum slots [65, 128]: chunk p closes window p-1
      (start=False) and opens window p (start=True).  After compile,
      _dedup_ldweights deletes the second, redundant v LDWEIGHTS of each
      close/open pair straight out of the scheduled instruction stream.
  DVE: evacuate psum slot banks [65, 4 windows, 128] -> fp16 sbuf; DMA out
      every 8 windows.

PSUM budget (8 banks): 3 score buffers x [128, 1024] f32 (2 banks each) for
exp lookahead + 2 out-slot banks.

Accuracy: ~7e-4 relative (fp16 operands and fp16 unnormalized output; exact
fp32 PSUM accumulation; host-side fp32 division).
"""

import os
import sys

for _p in ("/opt/trn_rl_repo", "/opt/pypackages"):
    if os.path.isdir(_p) and _p not in sys.path:
        sys.path.append(_p)

import numpy as np

import concourse.mybir as mybir
import concourse.tile as tile
from concourse import bacc
from concourse.bass_utils import run_bass_kernel_spmd

B, H, N, D = 8, 8, 4096, 64
WS = 128                 # window size
W = N // WS              # 32 windows
C = W + 1                # 33 key chunks incl. pad chunk
NC = 8                   # cores
HPC = (B * H) // NC      # 8 heads per core
PAIRS = HPC // 2         # 4 head pairs per core
SCALE = float(D) ** -0.5
EXP_BIAS = -3.5          # exp(x*SCALE + EXP_BIAS): cancels in P/l, keeps fp16 range

MM_DT = mybir.dt.float16
GROUP = 2                # key chunks per exp batch (h-block must stay 512-col
                         # = one psum bank aligned, so GROUP must be even)
EVW = 4                  # windows per psum out bank
DMW = 8                  # windows per out DMA (2 banks per staging tile)
VP = 72                  # v chunk padded to 72 cols: 144 B rows keep the
                         # 65-col LDWEIGHTS slices 16 B-aligned in SBUF

_NC_CACHE = {}


def build_nc(pairs=PAIRS, w=W):
    c = w + 1
    n = w * WS
    nc = bacc.Bacc("TRN2", target_bir_lowering=False)
    qT = nc.dram_tensor("qT", [pairs, 128, n], MM_DT, kind="ExternalInput")
    kT = nc.dram_tensor("kT", [pairs, 128, c * WS], MM_DT, kind="ExternalInput")
    vv = nc.dram_tensor("v", [2 * pairs, 128, c, VP], MM_DT, kind="ExternalInput")
    out = nc.dram_tensor("out", [2 * pairs, D + 1, n], MM_DT,
                         kind="ExternalOutput")

    f32 = mybir.dt.float32
    ch = c // 2
    Exp = mybir.ActivationFunctionType.Exp

    with tile.TileContext(nc) as tc:
        with (
            tc.tile_pool(name="cst", bufs=1) as cst_pool,
            tc.tile_pool(name="qk", bufs=2) as qk_pool,
            tc.tile_pool(name="vp", bufs=4) as v_pool,
            tc.tile_pool(name="pt", bufs=6) as pt_pool,
            tc.tile_pool(name="st", bufs=6) as st_pool,
            tc.tile_pool(name="ps_s", bufs=3, space="PSUM") as ps_s,
            tc.tile_pool(name="ps_o", bufs=2, space="PSUM") as ps_o,
        ):
            bias_t = cst_pool.tile([128, 1], f32, tag="bias")
            nc.vector.memset(bias_t[:, :], EXP_BIAS)
            # trigger the one-time exp ACT_TABLE_LOAD during the input DMA wait
            warm_t = cst_pool.tile([128, 1], MM_DT, tag="warm")
            nc.scalar.activation(warm_t[:, :], bias_t[:, :], Exp,
                                 bias=bias_t[:, 0:1], scale=SCALE)

            for pair in range(pairs):
                qt = qk_pool.tile([128, n], MM_DT, tag="qT")
                kt = qk_pool.tile([128, c * WS], MM_DT, tag="kT")
                vts = [v_pool.tile([128, c, VP], MM_DT, tag="v",
                                   name=f"v_{pair}_{h}") for h in range(2)]

                NSL = 8 if pair == 0 else 4
                ck, cq = c * WS // NSL, n // NSL

                def load_slice(sl):
                    nc.sync.dma_start(kt[:, sl * ck:(sl + 1) * ck],
                                      kT[pair][:, sl * ck:(sl + 1) * ck])
                    nc.sync.dma_start(qt[:, sl * cq:(sl + 1) * cq],
                                      qT[pair][:, sl * cq:(sl + 1) * cq])

                load_slice(0)
                # interleave v halves between input slices: HWDGE DMAs drain
                # FIFO per engine, so a monolithic v load would delay the
                # kt/qt slices that feed the next MM1s
                for h in range(2):
                    nc.sync.dma_start(vts[h][:, 0:ch], vv[2 * pair + h][:, 0:ch])
                load_slice(1)
                for h in range(2):
                    nc.sync.dma_start(vts[h][:, ch:], vv[2 * pair + h][:, ch:])
                for sl in range(2, NSL):
                    load_slice(sl)

                # per-head ring of psum out banks, each holding EVW windows
                banks = [dict(), dict()]   # h -> {bank_idx: psum tile}

                def get_bank(h, wi):
                    bi = wi // EVW
                    if bi not in banks[h]:
                        banks[h][bi] = ps_o.tile(
                            [D + 1, EVW, WS], f32, tag="out",
                            name=f"ob_{pair}_{h}_{bi}")
                    return banks[h][bi]

                stgs = [dict(), dict()]  # h -> {dma_idx: staging tile}

                def evac_bank(h, bi, nwin):
                    t = banks[h].pop(bi)
                    di, half = divmod(bi, DMW // EVW)
                    if di not in stgs[h]:
                        stgs[h][di] = st_pool.tile(
                            [D + 1, DMW, WS], MM_DT, tag="stg",
                            name=f"st_{pair}_{h}_{di}")
                    stg = stgs[h][di]
                    nc.vector.tensor_copy(
                        stg[:, half * EVW:half * EVW + nwin], t[:, 0:nwin])
                    if half * EVW + nwin == DMW or bi * EVW + nwin == w:
                        stgs[h].pop(di)
                        nc.sync.dma_start(
                            out[2 * pair + h][:, di * DMW * WS:
                                              (di * DMW + half * EVW + nwin)
                                              * WS],
                            stg[:, 0:half * EVW + nwin])

                groups = [list(range(g, min(g + GROUP, c)))
                          for g in range(0, c, GROUP)]
                pending_mm2 = None

                def do_mm2s(chunks, pt):
                    for s, p in enumerate(chunks):
                        for h in range(2):
                            col = h * (GROUP * 256) + s * 256
                            vav = vts[h][:, p, 0:D + 1]
                            if p >= 1:
                                # close window p-1 (self-keys contribution);
                                # emitted before the open so the open's bank
                                # recycle never waits on a later instruction
                                wi = p - 1
                                bank = get_bank(h, wi)
                                nc.tensor.matmul(
                                    bank[:, wi % EVW],
                                    vav, pt[:, col:col + WS],
                                    start=False, stop=True,
                                )
                                if (wi + 1) % EVW == 0 or wi == w - 1:
                                    evac_bank(h, wi // EVW, wi % EVW + 1)
                            if p <= w - 1:
                                # open window p (its prev-keys contribution)
                                colB = col + (WS if p >= 1 else 0)
                                bank = get_bank(h, p)
                                nc.tensor.matmul(
                                    bank[:, p % EVW],
                                    vav, pt[:, colB:colB + WS],
                                    start=True, stop=False,
                                )

                for chunks in groups:
                    ps = ps_s.tile([128, GROUP * 2 * 256], f32, tag="scores")
                    runs = []  # written (col, n) regions
                    for s, p in enumerate(chunks):
                        qlo = max(0, (p - 1) * WS)
                        qhi = min(n, (p + 1) * WS)
                        if p == 0:
                            qhi = 2 * WS   # fill the slot: keeps the exp run
                        nq = qhi - qlo     # contiguous (upper half unused)
                        for h in range(2):
                            col = h * (GROUP * 256) + s * 256
                            nc.tensor.matmul(
                                ps[:, col:col + nq],
                                kt[64 * h:64 * h + 64, p * WS:(p + 1) * WS],
                                qt[64 * h:64 * h + 64, qlo:qhi],
                                start=True, stop=True,
                            )
                            runs.append((col, nq))
                    pt = pt_pool.tile([128, GROUP * 2 * 256], MM_DT, tag="pt")
                    merged = []
                    for rcol, rn in sorted(runs):
                        if merged and merged[-1][0] + merged[-1][1] == rcol:
                            merged[-1][1] += rn
                        else:
                            merged.append([rcol, rn])
                    for rcol, rn in merged:
                        nc.scalar.activation(pt[:, rcol:rcol + rn],
                                             ps[:, rcol:rcol + rn],
                                             Exp, bias=bias_t[:, 0:1],
                                             scale=SCALE)
                    if pending_mm2 is not None:
                        do_mm2s(*pending_mm2)
                    pending_mm2 = (chunks, pt)
                if pending_mm2 is not None:
                    do_mm2s(*pending_mm2)
                    pending_mm2 = None

    nc.compile()
    _dedup_ldweights(nc)
    return nc


def _dedup_ldweights(nc):
    """Drop back-to-back identical PE weight loads.

    The close/open MM2 matmuls of a chunk share the same stationary v tile
    but legalization emits one InstLdweights per matmul.  After scheduling,
    wherever two consecutive PE InstLdweights have identical physical access
    patterns (only non-LDW PE instructions between them) the second load is
    redundant - the weights are still resident in the array.  Only sync-free
    duplicates are dropped so no semaphore bookkeeping changes.
    """
    removed = 0
    pe = mybir.EngineType.PE
    for fn in nc.m.functions:
        for blk in fn.blocks:
            il = blk.instructions
            keep = []
            last_key = None
            for inst in il:
                if getattr(inst, "engine", None) == pe:
                    tname = type(inst).__name__
                    if tname == "InstLdweights":
                        key = (repr(inst.ins), inst.tile_position,
                               inst.perf_mode, inst.is_transpose)
                        si = inst.sync_info
                        clean = si is None or (not si.on_wait
                                               and not si.on_update)
                        if key == last_key and clean:
                            removed += 1
                            continue
                        last_key = key
                keep.append(inst)
            if removed:
                il[:] = keep
    if removed:
        print(f"[kernel] deduped {removed} redundant LDWEIGHTS", file=sys.stderr)
    return removed


def _get_nc():
    if "nc" not in _NC_CACHE:
        _NC_CACHE["nc"] = build_nc()
    return _NC_CACHE["nc"]


def _prep_core(qf, kf, vf, lo):
    """Build one core's input dict from flat [64, 4096, 64] fp32 arrays."""
    q8 = qf[lo:lo + HPC]                      # [8, 4096, 64]
    k8 = kf[lo:lo + HPC]
    v8 = vf[lo:lo + HPC]

    qT = np.ascontiguousarray(q8.transpose(0, 2, 1)).reshape(PAIRS, 128, N)
    qT = qT.astype(np.float16)

    pad = np.full((HPC, WS, D), -1.0, dtype=np.float32)
    kp = np.concatenate([pad, k8], axis=1)    # [8, 4224, 64]
    kT = np.ascontiguousarray(kp.transpose(0, 2, 1)).reshape(PAIRS, 128, C * WS)
    kT = kT.astype(np.float16)

    vp = np.concatenate([pad, v8], axis=1)    # [8, 4224, 64]
    ones = np.ones((HPC, C * WS, 1), dtype=np.float32)
    zpad = np.zeros((HPC, C * WS, VP - D - 1), dtype=np.float32)
    va = np.concatenate([vp, ones, zpad], axis=2)   # [8, 4224, 72]
    va = va.reshape(HPC, C, WS, VP).transpose(0, 2, 1, 3)  # [8, 128, 33, 72]
    va = np.ascontiguousarray(va).astype(np.float16)

    return {"qT": qT, "kT": kT, "v": va}


def kernel(q, k, v):
    q = np.asarray(q, dtype=np.float32)
    k = np.asarray(k, dtype=np.float32)
    v = np.asarray(v, dtype=np.float32)
    qf = q.reshape(B * H, N, D)
    kf = k.reshape(B * H, N, D)
    vf = v.reshape(B * H, N, D)

    nc = _get_nc()
    in_maps = [_prep_core(qf, kf, vf, HPC * c) for c in range(NC)]
    res = run_bass_kernel_spmd(nc, in_maps, core_ids=list(range(NC)))

    outs = []
    for c in range(NC):
        o = res.results[c]["out"].astype(np.float32)   # [8, 65, 4096]
        o = o[:, :D, :] / o[:, D:D + 1, :]             # normalize by l row
        outs.append(o.transpose(0, 2, 1))              # [8, 4096, 64]
    return np.concatenate(outs, axis=0).reshape(B, H, N, D).astype(np.float32)


if __name__ == "__main__":
    rng = np.random.default_rng(0)
    q = rng.standard_normal((B, H, N, D), dtype=np.float32)
    k = rng.standard_normal((B, H, N, D), dtype=np.float32)
    v = rng.standard_normal((B, H, N, D), dtype=np.float32)
    o = kernel(q, k, v)
    print("out", o.shape, o.dtype, float(np.abs(o).max()))


# revision 44
# speedup vs baseline: 1.1266x; 1.0836x over previous
"""Local (windowed) attention Trainium2 Bass kernel.

Problem: q,k,v [8, 8, 4096, 64] fp32; window 128, look_backward 1, pad -1.0.
out[b,h,w,i,:] = softmax(scale * q_wi . [k_{w-1}; k_w]) @ [v_{w-1}; v_w]
(with window -1 = all -1.0 pad values, which DO enter the softmax).

Sharding: data-parallel over flat batch*heads (64) -> 8 heads per core.

Per-core layouts (prepared host-side):
  qT : [4, 128, 4096]  float16 - head pair stacked on partitions (d=64 each),
                                 free axis = 4096 queries (d-major transposed)
  kT : [4, 128, 4224]  float16 - same, with one pad chunk (128 keys of -1.0)
                                 prepended -> 33 chunks of 128 keys
  v  : [8, 128, 33, 72] float16 - per head; partition = key-within-chunk,
                                 pad chunk prepended; col 64 = 1.0 (ones
                                 column yields softmax denominator l); cols
                                 65..71 zero pad for 16 B row alignment
  out: [8, 65, 4096] float16 - TRANSPOSED unnormalized output: rows 0..63 =
                               sum_j P[j,q] v[j,:], row 64 = l[q].  The final
                               division by l happens on the host.

Device pipeline per head pair, per key chunk p (0..32):
  MM1 (fp16): scoresT[j, i] for the <=2 windows attending chunk p
              lhsT = kT chunk [64,128], rhs = qT slice [64,256];
              heads of a pair alternate PE row groups (base partition 0/64)
              so LDWEIGHTS pulls ahead under the other head's matmul and the
              row-tiled matmuls run concurrently; each PSUM bank only ever
              sees one weight base partition (mixing row-group bases within
              a bank hard-crashes the device).
  ACT exp (scale=1/8, bias=-3.5) psum -> fp16 P tiles, one [128, 1024]
      ACTIVATE per 2-chunk group (both heads contiguous).  The constant
      bias keeps exp sums in fp16 range; it cancels in the final P/l
      division.  The exp table is pre-loaded during the initial DMA wait.
  MM2 (fp16), operands swapped vs the score layout: stationary = v_aug
      chunk [128 keys, 65] (65-column LDWEIGHTS instead of 128-column P
      loads), moving = P columns.  Output is out^T [65, queries] in
      per-window psum slots [65, 128]: chunk p closes window p-1
      (start=False) and opens window p (start=True).  After compile,
      _dedup_ldweights deletes the second, redundant v LDWEIGHTS of each
      close/open pair straight out of the scheduled instruction stream.
  DVE: evacuate psum slot banks [65, 4 windows, 128] -> fp16 sbuf; DMA out
      every 8 windows.

PSUM budget (8 banks): 3 score buffers x [128, 1024] f32 (2 banks each) for
exp lookahead + 2 out-slot banks.

Accuracy: ~7e-4 relative (fp16 operands and fp16 unnormalized output; exact
fp32 PSUM accumulation; host-side fp32 division).
"""

import os
import sys

for _p in ("/opt/trn_rl_repo", "/opt/pypackages"):
    if os.path.isdir(_p) and _p not in sys.path:
        sys.path.append(_p)

import numpy as np

import concourse.mybir as mybir
import concourse.tile as tile
from concourse import bacc
from concourse.bass_utils import run_bass_kernel_spmd

B, H, N, D = 8, 8, 4096, 64
WS = 128                 # window size
W = N // WS              # 32 windows
C = W + 1                # 33 key chunks incl. pad chunk
NC = 8                   # cores
HPC = (B * H) // NC      # 8 heads per core
PAIRS = HPC // 2         # 4 head pairs per core
SCALE = float(D) ** -0.5
EXP_BIAS = -3.5          # exp(x*SCALE + EXP_BIAS): cancels in P/l, keeps fp16 range

MM_DT = mybir.dt.float16
GROUP = 2                # key chunks per exp batch (h-block must stay 512-col
                         # = one psum bank aligned, so GROUP must be even)
EVW = 4                  # windows per psum out bank
DMW = 8                  # windows per out DMA (2 banks per staging tile)
VP = 72                  # v chunk padded to 72 cols: 144 B rows keep the
                         # 65-col LDWEIGHTS slices 16 B-aligned in SBUF

_NC_CACHE = {}


def build_nc(pairs=PAIRS, w=W):
    c = w + 1
    n = w * WS
    nc = bacc.Bacc("TRN2", target_bir_lowering=False)
    qT = nc.dram_tensor("qT", [pairs, 128, n], MM_DT, kind="ExternalInput")
    kT = nc.dram_tensor("kT", [pairs, 128, c * WS], MM_DT, kind="ExternalInput")
    vv = nc.dram_tensor("v", [2 * pairs, 128, c, VP], MM_DT, kind="ExternalInput")
    out = nc.dram_tensor("out", [2 * pairs, D + 1, n], MM_DT,
                         kind="ExternalOutput")

    f32 = mybir.dt.float32
    ch = c // 2
    Exp = mybir.ActivationFunctionType.Exp

    with tile.TileContext(nc) as tc:
        with (
            tc.tile_pool(name="cst", bufs=1) as cst_pool,
            tc.tile_pool(name="qk", bufs=2) as qk_pool,
            tc.tile_pool(name="vp", bufs=4) as v_pool,
            tc.tile_pool(name="pt", bufs=6) as pt_pool,
            tc.tile_pool(name="st", bufs=6) as st_pool,
            tc.tile_pool(name="ps_s", bufs=2, space="PSUM") as ps_s,
            tc.tile_pool(name="ps_o", bufs=3, space="PSUM") as ps_o,
        ):
            bias_t = cst_pool.tile([128, 1], f32, tag="bias")
            nc.vector.memset(bias_t[:, :], EXP_BIAS)
            # trigger the one-time exp ACT_TABLE_LOAD during the input DMA wait
            warm_t = cst_pool.tile([128, 1], MM_DT, tag="warm")
            nc.scalar.activation(warm_t[:, :], bias_t[:, :], Exp,
                                 bias=bias_t[:, 0:1], scale=SCALE)

            for pair in range(pairs):
                qt = qk_pool.tile([128, n], MM_DT, tag="qT")
                kt = qk_pool.tile([128, c * WS], MM_DT, tag="kT")
                vts = [v_pool.tile([128, c, VP], MM_DT, tag="v",
                                   name=f"v_{pair}_{h}") for h in range(2)]

                NSL = 8 if pair == 0 else 4
                ck, cq = c * WS // NSL, n // NSL

                def load_slice(sl):
                    nc.sync.dma_start(kt[:, sl * ck:(sl + 1) * ck],
                                      kT[pair][:, sl * ck:(sl + 1) * ck])
                    nc.sync.dma_start(qt[:, sl * cq:(sl + 1) * cq],
                                      qT[pair][:, sl * cq:(sl + 1) * cq])

                load_slice(0)
                # interleave v halves between input slices: HWDGE DMAs drain
                # FIFO per engine, so a monolithic v load would delay the
                # kt/qt slices that feed the next MM1s
                for h in range(2):
                    nc.sync.dma_start(vts[h][:, 0:ch], vv[2 * pair + h][:, 0:ch])
                load_slice(1)
                for h in range(2):
                    nc.sync.dma_start(vts[h][:, ch:], vv[2 * pair + h][:, ch:])
                for sl in range(2, NSL):
                    load_slice(sl)

                # per-head ring of psum out banks, each holding EVW windows
                banks = [dict(), dict()]   # h -> {bank_idx: psum tile}

                def get_bank(h, wi):
                    bi = wi // EVW
                    if bi not in banks[h]:
                        banks[h][bi] = ps_o.tile(
                            [D + 1, EVW, WS], f32, tag="out",
                            name=f"ob_{pair}_{h}_{bi}")
                    return banks[h][bi]

                stgs = [dict(), dict()]  # h -> {dma_idx: staging tile}

                def evac_bank(h, bi, nwin):
                    t = banks[h].pop(bi)
                    di, half = divmod(bi, DMW // EVW)
                    if di not in stgs[h]:
                        stgs[h][di] = st_pool.tile(
                            [D + 1, DMW, WS], MM_DT, tag="stg",
                            name=f"st_{pair}_{h}_{di}")
                    stg = stgs[h][di]
                    nc.vector.tensor_copy(
                        stg[:, half * EVW:half * EVW + nwin], t[:, 0:nwin])
                    if half * EVW + nwin == DMW or bi * EVW + nwin == w:
                        stgs[h].pop(di)
                        nc.sync.dma_start(
                            out[2 * pair + h][:, di * DMW * WS:
                                              (di * DMW + half * EVW + nwin)
                                              * WS],
                            stg[:, 0:half * EVW + nwin])

                groups = [list(range(g, min(g + GROUP, c)))
                          for g in range(0, c, GROUP)]
                pending_mm2 = None

                def do_mm2s(chunks, pt):
                    for s, p in enumerate(chunks):
                        for h in range(2):
                            col = h * (GROUP * 256) + s * 256
                            vav = vts[h][:, p, 0:D + 1]
                            if p >= 1:
                                # close window p-1 (self-keys contribution);
                                # emitted before the open so the open's bank
                                # recycle never waits on a later instruction
                                wi = p - 1
                                bank = get_bank(h, wi)
                                nc.tensor.matmul(
                                    bank[:, wi % EVW],
                                    vav, pt[:, col:col + WS],
                                    start=False, stop=True,
                                )
                                if (wi + 1) % EVW == 0 or wi == w - 1:
                                    evac_bank(h, wi // EVW, wi % EVW + 1)
                            if p <= w - 1:
                                # open window p (its prev-keys contribution)
                                colB = col + (WS if p >= 1 else 0)
                                bank = get_bank(h, p)
                                nc.tensor.matmul(
                                    bank[:, p % EVW],
                                    vav, pt[:, colB:colB + WS],
                                    start=True, stop=False,
                                )

                for chunks in groups:
                    ps = ps_s.tile([128, GROUP * 2 * 256], f32, tag="scores")
                    runs = []  # written (col, n) regions
                    for s, p in enumerate(chunks):
                        qlo = max(0, (p - 1) * WS)
                        qhi = min(n, (p + 1) * WS)
                        if p == 0:
                            qhi = 2 * WS   # fill the slot: keeps the exp run
                        nq = qhi - qlo     # contiguous (upper half unused)
                        for h in range(2):
                            col = h * (GROUP * 256) + s * 256
                            nc.tensor.matmul(
                                ps[:, col:col + nq],
                                kt[64 * h:64 * h + 64, p * WS:(p + 1) * WS],
                                qt[64 * h:64 * h + 64, qlo:qhi],
                                start=True, stop=True,
                            )
                            runs.append((col, nq))
                    pt = pt_pool.tile([128, GROUP * 2 * 256], MM_DT, tag="pt")
                    merged = []
                    for rcol, rn in sorted(runs):
                        if merged and merged[-1][0] + merged[-1][1] == rcol:
                            merged[-1][1] += rn
                        else:
                            merged.append([rcol, rn])
                    for rcol, rn in merged:
                        nc.scalar.activation(pt[:, rcol:rcol + rn],
                                             ps[:, rcol:rcol + rn],
                                             Exp, bias=bias_t[:, 0:1],
                                             scale=SCALE)
                    if pending_mm2 is not None:
                        do_mm2s(*pending_mm2)
                    pending_mm2 = (chunks, pt)
                if pending_mm2 is not None:
                    do_mm2s(*pending_mm2)
                    pending_mm2 = None

    nc.compile()
    _dedup_ldweights(nc)
    return nc


def _dedup_ldweights(nc):
    """Drop back-to-back identical PE weight loads.

    The close/open MM2 matmuls of a chunk share the same stationary v tile
    but legalization emits one InstLdweights per matmul.  After scheduling,
    wherever two consecutive PE InstLdweights have identical physical access
    patterns (only non-LDW PE instructions between them) the second load is
    redundant - the weights are still resident in the array.  Only sync-free
    duplicates are dropped so no semaphore bookkeeping changes.
    """
    removed = 0
    pe = mybir.EngineType.PE
    for fn in nc.m.functions:
        for blk in fn.blocks:
            il = blk.instructions
            keep = []
            last_key = None
            for inst in il:
                if getattr(inst, "engine", None) == pe:
                    tname = type(inst).__name__
                    if tname == "InstLdweights":
                        key = (repr(inst.ins), inst.tile_position,
                               inst.perf_mode, inst.is_transpose)
                        si = inst.sync_info
                        clean = si is None or (not si.on_wait
                                               and not si.on_update)
                        if key == last_key and clean:
                            removed += 1
                            continue
                        last_key = key
                keep.append(inst)
            if removed:
                il[:] = keep
    if removed:
        print(f"[kernel] deduped {removed} redundant LDWEIGHTS", file=sys.stderr)
    return removed


def _get_nc():
    if "nc" not in _NC_CACHE:
        _NC_CACHE["nc"] = build_nc()
    return _NC_CACHE["nc"]


def _prep_core(qf, kf, vf, lo):
    """Build one core's input dict from flat [64, 4096, 64] fp32 arrays."""
    q8 = qf[lo:lo + HPC]                      # [8, 4096, 64]
    k8 = kf[lo:lo + HPC]
    v8 = vf[lo:lo + HPC]

    qT = np.ascontiguousarray(q8.transpose(0, 2, 1)).reshape(PAIRS, 128, N)
    qT = qT.astype(np.float16)

    pad = np.full((HPC, WS, D), -1.0, dtype=np.float32)
    kp = np.concatenate([pad, k8], axis=1)    # [8, 4224, 64]
    kT = np.ascontiguousarray(kp.transpose(0, 2, 1)).reshape(PAIRS, 128, C * WS)
    kT = kT.astype(np.float16)

    vp = np.concatenate([pad, v8], axis=1)    # [8, 4224, 64]
    ones = np.ones((HPC, C * WS, 1), dtype=np.float32)
    zpad = np.zeros((HPC, C * WS, VP - D - 1), dtype=np.float32)
    va = np.concatenate([vp, ones, zpad], axis=2)   # [8, 4224, 72]
    va = va.reshape(HPC, C, WS, VP).transpose(0, 2, 1, 3)  # [8, 128, 33, 72]
    va = np.ascontiguousarray(va).astype(np.float16)

    return {"qT": qT, "kT": kT, "v": va}


def kernel(q, k, v):
    q = np.asarray(q, dtype=np.float32)
    k = np.asarray(k, dtype=np.float32)
    v = np.asarray(v, dtype=np.float32)
    qf = q.reshape(B * H, N, D)
    kf = k.reshape(B * H, N, D)
    vf = v.reshape(B * H, N, D)

    nc = _get_nc()
    in_maps = [_prep_core(qf, kf, vf, HPC * c) for c in range(NC)]
    res = run_bass_kernel_spmd(nc, in_maps, core_ids=list(range(NC)))

    outs = []
    for c in range(NC):
        o = res.results[c]["out"].astype(np.float32)   # [8, 65, 4096]
        o = o[:, :D, :] / o[:, D:D + 1, :]             # normalize by l row
        outs.append(o.transpose(0, 2, 1))              # [8, 4096, 64]
    return np.concatenate(outs, axis=0).reshape(B, H, N, D).astype(np.float32)


if __name__ == "__main__":
    rng = np.random.default_rng(0)
    q = rng.standard_normal((B, H, N, D), dtype=np.float32)
    k = rng.standard_normal((B, H, N, D), dtype=np.float32)
    v = rng.standard_normal((B, H, N, D), dtype=np.float32)
    o = kernel(q, k, v)
    print("out", o.shape, o.dtype, float(np.abs(o).max()))
